# revision 16
# baseline (speedup 1.0000x reference)
"""BiAttentionLayer Trainium2 kernel (Bass/Tile), data-parallel over batch N.

Full inputs:  H [64,1024,200], U [64,64,200], c_mask [64,1024],
              q_mask [64,64], w [600], b []
Full output:  G [64,1024,800] = concat([H, U_, H*U_, H*H_], -1)

Sharding: batch rows 8 per core across 8 NeuronCores; masks/w/b replicated.

v3 design (bf16 matmuls + big cast-DMAs + min instruction count):

  masked_softmax(v,m) == exp(v*m - 100)*m / sum_j(...)  (normalizer cancels)
  The per-t term S1 = H@w_h cancels inside the C2Q softmax (uniform factor
  exp(S1) over unmasked lanes), so S1 is EXCLUDED from the softmax logits and
  instead emitted as PSUM column 64 of each chunk (w_h appended to the
  S-matmul rhs); the Q2C branch uses rt = exp(S1)*max_j(e')*cm.
  Logits: St[t,j] = (S_core + S2 + b + 100)*qm accumulated as
    mm1/mm2: H^T-chunk x (U^T*w_hu*qm | w_h)   (d contracted, 128+72 split)
    mm3:     rank-1 ones2 x [ (S2+b)*qm ; 100*qm ]  (two separate bf16 rows:
             100.0 is exact in bf16; (S2+b+100) rounded to bf16 would
             perturb logits by ~0.25 and distort softmax weights ~30%)
  e' = exp(St - 100) in one ACT op per half-row [128, 4*65].
  U_ = (e' @ [U | 1]) gives numerator and denominator in one matmul.

Layout: chunk e of a batch row holds t = 8p + e (p = SBUF partition), so the
H load is one contiguous cast-DMA per row (6.4KB per-partition descriptors),
and G is stored as two DMAs per row: cols 0:200 cast-stored directly from
the bf16 H tile (SWDGE), cols 200:800 stored fp32 via HWDGE on the idle
Sync queue.  All PE operands bf16 (1 cyc/row vs 4 fp32).
"""

import os
import sys

for _p in ("/opt/trn_rl_repo", "/root/.axon_site/_ro/trn_rl_repo"):
    if os.path.isdir(_p) and _p not in sys.path:
        sys.path.insert(0, _p)

import numpy as np

import concourse.bass as bass
import concourse.tile as tile
from concourse import mybir
from concourse.masks import make_identity

N_CORES = 8
N_FULL = 64
B = N_FULL // N_CORES          # batch rows per core
T = 1024
J = 64
D2 = 200
DG = 4 * D2                    # 800
DGR = 3 * D2                   # U_|HU|HH part of G kept in SBUF
E = 8                          # chunks per row; chunk e holds t = 8p+e
JS = J + 1                     # 64 logit cols + S1 col per chunk
K1, K2 = 128, D2 - 128         # contraction split 128 + 72
NEG_SOFT = 100.0               # exp(x - 100): masked lanes underflow to 0

FP = mybir.dt.float32
BF = mybir.dt.bfloat16
MULT = mybir.AluOpType.mult
ADD = mybir.AluOpType.add
AXX = mybir.AxisListType.X
EXP = mybir.ActivationFunctionType.Exp
COPYF = mybir.ActivationFunctionType.Copy


def _split_overwide_waits(nc, max_waits=1):
    """This walrus build only encodes one semaphore wait per instruction;
    hoist extra waits onto no-ops just before the offending instruction."""
    for bb in nc.m.functions[0].blocks:
        i = 0
        while i < len(bb.instructions):
            ins = bb.instructions[i]
            si = getattr(ins, "sync_info", None)
            if si is not None and si.on_wait is not None and len(si.on_wait) > max_waits:
                waits = list(si.on_wait)
                si.on_wait = waits[-max_waits:]
                rest = waits[:-max_waits]
                k = 0
                while rest:
                    chunk, rest = rest[:max_waits], rest[max_waits:]
                    nop = mybir.InstNoOp(
                        name=f"{ins.name}-wsplit{k}",
                        engine=ins.engine,
                        bass_nofuse=True,
                        sync_info=mybir.SyncInfo(on_wait=chunk, on_update=[]),
                    )
                    bb.instructions.insert(i, nop)
                    i += 1
                    k += 1
            i += 1


def build_program(split_waits=True):
    nc = bass.Bass()

    H_d = nc.dram_tensor("H", [B, T, D2], FP, kind="ExternalInput")
    U_d = nc.dram_tensor("U", [B, J, D2], FP, kind="ExternalInput")
    cm_d = nc.dram_tensor("c_mask", [B, T], FP, kind="ExternalInput")
    qm_d = nc.dram_tensor("q_mask", [B, J], FP, kind="ExternalInput")
    w_d = nc.dram_tensor("w", [3 * D2], FP, kind="ExternalInput")
    b_d = nc.dram_tensor("b", [1, 1], FP, kind="ExternalInput")
    G_d = nc.dram_tensor("G", [B, T, DG], FP, kind="ExternalOutput")

    with tile.TileContext(nc) as tc:
        with (
            tc.tile_pool(name="const", bufs=1) as constp,
            tc.tile_pool(name="row", bufs=2) as rowp,
            tc.tile_pool(name="hrow", bufs=3) as hrowp,
            tc.tile_pool(name="chunk", bufs=6) as chp,
            tc.tile_pool(name="gbuf", bufs=3) as gp,
            tc.tile_pool(name="ps_tr", bufs=3, space="PSUM") as ps_trp,
            tc.tile_pool(name="ps_s", bufs=2, space="PSUM") as ps_sp,
            tc.tile_pool(name="ps_u", bufs=2, space="PSUM") as ps_up,
            tc.tile_pool(name="ps_sm", bufs=1, space="PSUM") as ps_smp,
        ):
            # ---- constants ----
            identb = constp.tile([128, 128], BF)
            make_identity(nc, identb)
            ident64 = constp.tile([64, 64], FP)
            make_identity(nc, ident64)
            ones_row = constp.tile([1, 128], BF)
            nc.vector.memset(ones_row, 1.0)
            ones_col = constp.tile([128, 1], BF)
            nc.vector.memset(ones_col, 1.0)
            ones2 = constp.tile([2, 128], BF)
            nc.vector.memset(ones2, 1.0)
            negc = constp.tile([128, 1], FP)
            nc.vector.memset(negc, -NEG_SOFT)
            b_sb = constp.tile([1, 1], FP)
            nc.gpsimd.dma_start(out=b_sb, in_=b_d[:, :])
            # w splits: wh/wu feed matmul rhs -> bf16 (cast on DMA);
            # whu feeds a DVE scalar operand -> fp32.
            wh1 = constp.tile([K1, 1], BF)
            wh2 = constp.tile([K2, 1], BF)
            wu1 = constp.tile([K1, 1], BF)
            wu2 = constp.tile([K2, 1], BF)
            whu1 = constp.tile([K1, 1], FP)
            whu2 = constp.tile([K2, 1], FP)
            for sb, lo in ((wh1, 0), (wh2, K1), (wu1, D2), (wu2, D2 + K1),
                           (whu1, 2 * D2), (whu2, 2 * D2 + K1)):
                n = sb.shape[0]
                nc.gpsimd.dma_start(out=sb, in_=w_d[lo:lo + n].unsqueeze(1))

            def row_setup(r):
                st = {"r": r}
                U_sb = rowp.tile([J, D2], FP, tag="usb")
                nc.sync.dma_start(out=U_sb, in_=U_d[r])
                qm_b = rowp.tile([128, J], FP, tag="qmb")
                nc.gpsimd.dma_start(out=qm_b, in_=qm_d[r].partition_broadcast(128))
                cm_t = rowp.tile([128, E], FP, tag="cmt")
                nc.gpsimd.dma_start(
                    out=cm_t, in_=cm_d[r].rearrange("(p e) -> p e", p=128)
                )
                qm_bh = rowp.tile([128, J], BF, tag="qmbh")
                nc.gpsimd.dma_start(
                    out=qm_bh, in_=qm_d[r].partition_broadcast(128)
                )
                Hb = hrowp.tile([128, E * D2], BF, tag="hb")
                nc.gpsimd.dma_start(
                    out=Hb, in_=H_d[r].rearrange("(p e) d -> p (e d)", p=128)
                )
                # H part of G: cast-store straight from the bf16 H tile
                nc.gpsimd.dma_start(
                    out=G_d[r][:, 0:D2].rearrange("(p e) d -> p e d", p=128),
                    in_=Hb.rearrange("p (e d) -> p e d", d=D2),
                )

                # U in bf16 with a ones column (denominator trick)
                U_r = rowp.tile([J, D2 + 1], BF, tag="ur")
                nc.vector.tensor_copy(out=U_r[:, 0:D2], in_=U_sb)
                nc.gpsimd.memset(U_r[:, D2:D2 + 1], 1.0)

                # U^T (two d-blocks side by side) -> ut [128, 128]
                tru = ps_trp.tile([128, 256], BF, tag="tr", name="tru")
                nc.tensor.transpose(tru[:, 0:J], U_r[:, 0:K1], identb[0:J, 0:J])
                nc.tensor.transpose(
                    tru[0:K2, J:2 * J], U_r[:, K1:D2], identb[0:J, 0:J]
                )
                ut = rowp.tile([128, 2 * J], BF, tag="ut")
                nc.vector.tensor_copy(out=ut, in_=tru[:, 0:2 * J])

                # S-matmul rhs: [U^T*w_hu*qm | w_h] per d-block
                uwq1 = rowp.tile([K1, JS], BF, tag="uwq1")
                uwq2 = rowp.tile([K2, JS], BF, tag="uwq2")
                nc.vector.scalar_tensor_tensor(
                    out=uwq1[:, 0:J], in0=ut[:, 0:J], scalar=whu1[:, 0:1],
                    in1=qm_bh[0:K1, :], op0=MULT, op1=MULT,
                )
                nc.vector.tensor_copy(out=uwq1[:, J:JS], in_=wh1)
                nc.vector.scalar_tensor_tensor(
                    out=uwq2[:, 0:J], in0=ut[0:K2, J:2 * J], scalar=whu2[:, 0:1],
                    in1=qm_bh[0:K2, :], op0=MULT, op1=MULT,
                )
                nc.vector.tensor_copy(out=uwq2[:, J:JS], in_=wh2)

                # rank-1 rhs rows: [ (S2+b)*qm ; 100*qm ], S1 col = 0
                s2qm = rowp.tile([2, JS], BF, tag="s2qm")
                nc.vector.memset(s2qm, 0.0)
                ps2 = ps_smp.tile([J, 1], FP, tag="sm", name="ps2")
                nc.tensor.matmul(ps2, ut[:, 0:J], wu1, start=True, stop=False)
                nc.tensor.matmul(ps2, ut[0:K2, J:2 * J], wu2, start=False, stop=True)
                s2col = rowp.tile([J, 1], FP, tag="s2col")
                nc.vector.tensor_copy(out=s2col, in_=ps2)
                ps2r = ps_smp.tile([1, J], FP, tag="sm", name="ps2r")
                nc.tensor.transpose(ps2r, s2col, ident64)
                s2q = rowp.tile([1, J], BF, tag="s2q")
                nc.vector.scalar_tensor_tensor(
                    out=s2q, in0=ps2r, scalar=b_sb[:, 0:1],
                    in1=qm_b[0:1, :], op0=ADD, op1=MULT,
                )
                nc.sync.dma_start(out=s2qm[0:1, 0:J], in_=s2q)
                qm100 = rowp.tile([1, J], BF, tag="qm100")
                nc.vector.tensor_scalar_mul(
                    out=qm100, in0=qm_b[0:1, :], scalar1=NEG_SOFT
                )
                nc.sync.dma_start(out=s2qm[1:2, 0:J], in_=qm100)

                st["Hb"], st["U_r"], st["qm_b"], st["cm_t"] = Hb, U_r, qm_b, cm_t
                st["uwq1"], st["uwq2"], st["s2qm"] = uwq1, uwq2, s2qm
                st["g"] = gp.tile([128, E * DGR], FP, tag="g", name="g")
                st["ps_half"] = [None, None]
                st["e_half"] = [None, None]
                st["maxes"] = rowp.tile([128, E], FP, tag="maxes", name="maxes")
                st["expS1"] = rowp.tile([128, E], FP, tag="es1", name="expS1")
                st["ps_up"] = [None] * (E // 2)
                st["rp"] = [None] * (E // 2)
                return st

            def head(st, e):
                # logits chunk: St*qm in PSUM cols 0:64, S1 in col 64
                h2, idx = e // 4, e % 4
                if idx == 0:
                    st["ps_half"][h2] = ps_sp.tile(
                        [128, 4 * JS], FP, tag="srow", name="ps_half"
                    )
                ps = st["ps_half"][h2]
                Hb = st["Hb"]
                trc = ps_trp.tile([128, 256], BF, tag="tr", name="trc")
                nc.tensor.transpose(trc[:, 0:128], Hb[:, e * D2:e * D2 + K1], identb)
                nc.tensor.transpose(
                    trc[0:K2, 128:256], Hb[:, e * D2 + K1:(e + 1) * D2], identb
                )
                ht = chp.tile([128, 256], BF, tag="ht")
                nc.scalar.copy(out=ht, in_=trc)
                cols = slice(idx * JS, (idx + 1) * JS)
                nc.tensor.matmul(
                    ps[:, cols], ht[:, 0:128], st["uwq1"], start=True, stop=False
                )
                nc.tensor.matmul(
                    ps[:, cols], ht[0:K2, 128:256], st["uwq2"],
                    start=False, stop=False,
                )
                nc.tensor.matmul(
                    ps[:, cols], ones2, st["s2qm"], start=False, stop=True
                )

            def expblock(st, h2):
                # one ACT exp per half-row + S1 exp + per-chunk max
                ps = st["ps_half"][h2]
                e_half = chp.tile([128, 4 * JS], BF, tag="eh")
                st["e_half"][h2] = e_half
                nc.scalar.activation(
                    out=e_half, in_=ps, func=EXP, bias=negc[:, 0:1], scale=1.0
                )
                ps3 = ps.rearrange("p (c x) -> p c x", x=JS)
                nc.scalar.activation(
                    out=st["expS1"][:, h2 * 4:(h2 + 1) * 4],
                    in_=ps3[:, :, J:JS], func=EXP,
                )
                nc.vector.reduce_max(
                    st["maxes"][:, h2 * 4:(h2 + 1) * 4],
                    e_half.rearrange("p (c x) -> p c x", x=JS),
                    axis=AXX,
                )

            def pair(st, h):
                # chunks (2h, 2h+1): transpose e' pair, U_ matmuls + denom
                h2 = h // 2
                e_half = st["e_half"][h2]
                eTps = ps_trp.tile([J, 256], BF, tag="tr", name="eTps")
                for k in range(2):
                    idx = (2 * h + k) % 4
                    nc.tensor.transpose(
                        eTps[:, k * 128:(k + 1) * 128],
                        e_half[:, idx * JS:idx * JS + J], identb,
                    )
                eT = chp.tile([J, 256], BF, tag="eT")
                nc.vector.tensor_copy(out=eT, in_=eTps)
                ps_u = ps_up.tile([128, 2 * (D2 + 1)], FP, tag="up")
                st["ps_up"][h] = ps_u
                nc.tensor.matmul(
                    ps_u[:, 0:D2 + 1], eT[:, 0:128], st["U_r"],
                    start=True, stop=True,
                )
                nc.tensor.matmul(
                    ps_u[:, D2 + 1:2 * (D2 + 1)], eT[:, 128:256], st["U_r"],
                    start=True, stop=True,
                )
                rp = chp.tile([128, 2], FP, tag="rp")
                st["rp"][h] = rp
                nc.vector.reciprocal(
                    out=rp,
                    in_=ps_u.rearrange("p (c x) -> p c x", x=D2 + 1)[:, :, D2:D2 + 1],
                )

            def tail(st, c):
                h, half = c // 2, c % 2
                ps_u, rp, g = st["ps_up"][h], st["rp"][h], st["g"]
                lo = half * (D2 + 1)
                # U_ = (e'@U)/denom, normalization folded into the ACT copy
                nc.scalar.activation(
                    out=g[:, c * DGR:c * DGR + D2],
                    in_=ps_u[:, lo:lo + D2], func=COPYF,
                    scale=rp[:, half:half + 1],
                )
                nc.vector.tensor_tensor(
                    out=g[:, c * DGR + D2:c * DGR + 2 * D2],
                    in0=st["Hb"][:, c * D2:(c + 1) * D2],
                    in1=g[:, c * DGR:c * DGR + D2], op=MULT,
                )

            def rowend(st):
                # rt = exp(S1) * max_j(e') * cm ; H_ = (rt @ H) / sum(rt)
                rt = rowp.tile([128, E], FP, tag="rt")
                nc.vector.tensor_tensor(
                    out=rt, in0=st["maxes"], in1=st["expS1"], op=MULT
                )
                rtb = rowp.tile([128, E], BF, tag="rtb")
                nc.vector.tensor_tensor(
                    out=rtb, in0=rt, in1=st["cm_t"], op=MULT
                )
                Hb = st["Hb"]
                smt = ps_smp.tile([128, D2 + E], FP, tag="sm", name="smt")
                hbar = smt[0:1, 0:D2]
                for e in range(E):
                    nc.tensor.matmul(
                        hbar, rtb[:, e:e + 1], Hb[:, e * D2:(e + 1) * D2],
                        start=(e == 0), stop=(e == E - 1),
                    )
                rsum = smt[0:1, D2:D2 + E]
                nc.tensor.matmul(rsum, ones_col, rtb, start=True, stop=True)
                rsum_sb = rowp.tile([1, E], FP, tag="rsumsb")
                nc.vector.tensor_copy(out=rsum_sb, in_=rsum)
                rs = rowp.tile([1, 1], FP, tag="rs")
                nc.vector.reduce_sum(rs, rsum_sb, axis=AXX)
                nc.vector.reciprocal(out=rs, in_=rs)
                hbar_sb = rowp.tile([1, D2], BF, tag="hbarsb")
                nc.vector.tensor_scalar_mul(
                    out=hbar_sb, in0=hbar, scalar1=rs[:, 0:1]
                )
                ps_hb = smt[:, 0:D2]
                nc.tensor.matmul(ps_hb, ones_row, hbar_sb, start=True, stop=True)
                hb_sb = rowp.tile([128, D2], BF, tag="hbsb")
                nc.vector.tensor_copy(out=hb_sb, in_=ps_hb)
                st["hb_sb"] = hb_sb

            def rowfin(st, e):
                # H*H_ for one chunk (Pool keeps DVE free)
                g, Hb = st["g"], st["Hb"]
                nc.gpsimd.tensor_mul(
                    g[:, e * DGR + 2 * D2:e * DGR + 3 * D2],
                    Hb[:, e * D2:(e + 1) * D2], st["hb_sb"]
                )

            def rowstore(st, h2):
                # U_|HU|HH part of G, plain fp32 on the HWDGE sync queue;
                # stored in half-rows so the tail drains sooner
                es = slice(h2 * 4, (h2 + 1) * 4)
                nc.sync.dma_start(
                    out=G_d[st["r"]].rearrange(
                        "(p e) g -> p e g", p=128
                    )[:, es, D2:DG],
                    in_=st["g"].rearrange("p (e g) -> p e g", g=DGR)[:, es, :],
                )

            # ---- cross-row pipelined schedule ----
            states = [None] * B
            states[0] = row_setup(0)
            for e in range(E):
                head(states[0], e)
            for r in range(B):
                st = states[r]
                prev = states[r - 1] if r > 0 else None
                if r + 1 < B:
                    states[r + 1] = row_setup(r + 1)
                for h2 in range(2):
                    expblock(st, h2)
                    for h in (2 * h2, 2 * h2 + 1):
                        pair(st, h)
                        tail(st, 2 * h)
                        tail(st, 2 * h + 1)
                        if prev is not None:
                            rowfin(prev, 2 * h)
                            rowfin(prev, 2 * h + 1)
                        if r + 1 < B:
                            head(states[r + 1], 2 * h)
                            head(states[r + 1], 2 * h + 1)
                    if prev is not None:
                        rowstore(prev, h2)
                rowend(st)
            for h2 in range(2):
                for e in range(4 * h2, 4 * h2 + 4):
                    rowfin(states[B - 1], e)
                rowstore(states[B - 1], h2)

    if split_waits:
        _split_overwide_waits(nc)
    return nc


_NC_CACHE = None


def _get_nc():
    global _NC_CACHE
    if _NC_CACHE is None:
        _NC_CACHE = build_program()
    return _NC_CACHE


def run_sharded(inputs, trace=False):
    from concourse.bass_utils import run_bass_kernel_spmd

    H = np.ascontiguousarray(np.asarray(inputs["H"], dtype=np.float32))
    U = np.ascontiguousarray(np.asarray(inputs["U"], dtype=np.float32))
    cm = np.ascontiguousarray(np.asarray(inputs["c_mask"], dtype=np.float32))
    qm = np.ascontiguousarray(np.asarray(inputs["q_mask"], dtype=np.float32))
    w = np.ascontiguousarray(np.asarray(inputs["w"], dtype=np.float32))
    b = np.asarray(inputs["b"], dtype=np.float32).reshape(1, 1)

    nc = _get_nc()
    in_maps = []
    for c in range(N_CORES):
        s = slice(c * B, (c + 1) * B)
        in_maps.append(
            {"H": H[s], "U": U[s], "c_mask": cm[s], "q_mask": qm[s], "w": w, "b": b}
        )
    res = run_bass_kernel_spmd(
        nc, in_maps, core_ids=list(range(N_CORES)), trace=trace
    )
    G = np.concatenate([res.results[c]["G"] for c in range(N_CORES)], axis=0)
    return G, res


def kernel(H, U, c_mask, q_mask, w, b):
    G, _ = run_sharded(
        {"H": H, "U": U, "c_mask": c_mask, "q_mask": q_mask, "w": w, "b": b}
    )
    return G


# revision 18
# speedup vs baseline: 1.0661x; 1.0661x over previous
"""BiAttentionLayer Trainium2 kernel (Bass/Tile), data-parallel over batch N.

Full inputs:  H [64,1024,200], U [64,64,200], c_mask [64,1024],
              q_mask [64,64], w [600], b []
Full output:  G [64,1024,800] = concat([H, U_, H*U_, H*H_], -1)

Sharding: batch rows 8 per core across 8 NeuronCores; masks/w/b replicated.

v3 design (bf16 matmuls + big cast-DMAs + min instruction count):

  masked_softmax(v,m) == exp(v*m - 100)*m / sum_j(...)  (normalizer cancels)
  The per-t term S1 = H@w_h cancels inside the C2Q softmax (uniform factor
  exp(S1) over unmasked lanes), so S1 is EXCLUDED from the softmax logits and
  instead emitted as PSUM column 64 of each chunk (w_h appended to the
  S-matmul rhs); the Q2C branch uses rt = exp(S1)*max_j(e')*cm.
  Logits: St[t,j] = (S_core + S2 + b + 100)*qm accumulated as
    mm1/mm2: H^T-chunk x (U^T*w_hu*qm | w_h)   (d contracted, 128+72 split)
    mm3:     rank-1 ones2 x [ (S2+b)*qm ; 100*qm ]  (two separate bf16 rows:
             100.0 is exact in bf16; (S2+b+100) rounded to bf16 would
             perturb logits by ~0.25 and distort softmax weights ~30%)
  e' = exp(St - 100) in one ACT op per half-row [128, 4*65].
  U_ = (e' @ [U | 1]) gives numerator and denominator in one matmul.

Layout: chunk e of a batch row holds t = 8p + e (p = SBUF partition), so the
H load is one contiguous cast-DMA per row (6.4KB per-partition descriptors),
and G is stored as two DMAs per row: cols 0:200 cast-stored directly from
the bf16 H tile (SWDGE), cols 200:800 stored fp32 via HWDGE on the idle
Sync queue.  All PE operands bf16 (1 cyc/row vs 4 fp32).
"""

import os
import sys

for _p in ("/opt/trn_rl_repo", "/root/.axon_site/_ro/trn_rl_repo"):
    if os.path.isdir(_p) and _p not in sys.path:
        sys.path.insert(0, _p)

import numpy as np

import concourse.bass as bass
import concourse.tile as tile
from concourse import mybir
from concourse.masks import make_identity

N_CORES = 8
N_FULL = 64
B = N_FULL // N_CORES          # batch rows per core
T = 1024
J = 64
D2 = 200
DG = 4 * D2                    # 800
DGR = 3 * D2                   # U_|HU|HH part of G kept in SBUF
E = 8                          # chunks per row; chunk e holds t = 8p+e
JS = J + 1                     # 64 logit cols + S1 col per chunk
K1, K2 = 128, D2 - 128         # contraction split 128 + 72
NEG_SOFT = 100.0               # exp(x - 100): masked lanes underflow to 0

FP = mybir.dt.float32
BF = mybir.dt.bfloat16
MULT = mybir.AluOpType.mult
ADD = mybir.AluOpType.add
AXX = mybir.AxisListType.X
EXP = mybir.ActivationFunctionType.Exp
COPYF = mybir.ActivationFunctionType.Copy


def _split_overwide_waits(nc, max_waits=1):
    """This walrus build only encodes one semaphore wait per instruction;
    hoist extra waits onto no-ops just before the offending instruction."""
    for bb in nc.m.functions[0].blocks:
        i = 0
        while i < len(bb.instructions):
            ins = bb.instructions[i]
            si = getattr(ins, "sync_info", None)
            if si is not None and si.on_wait is not None and len(si.on_wait) > max_waits:
                waits = list(si.on_wait)
                si.on_wait = waits[-max_waits:]
                rest = waits[:-max_waits]
                k = 0
                while rest:
                    chunk, rest = rest[:max_waits], rest[max_waits:]
                    nop = mybir.InstNoOp(
                        name=f"{ins.name}-wsplit{k}",
                        engine=ins.engine,
                        bass_nofuse=True,
                        sync_info=mybir.SyncInfo(on_wait=chunk, on_update=[]),
                    )
                    bb.instructions.insert(i, nop)
                    i += 1
                    k += 1
            i += 1


def build_program(split_waits=True):
    nc = bass.Bass()

    H_d = nc.dram_tensor("H", [B, T, D2], FP, kind="ExternalInput")
    U_d = nc.dram_tensor("U", [B, J, D2], FP, kind="ExternalInput")
    cm_d = nc.dram_tensor("c_mask", [B, T], FP, kind="ExternalInput")
    qm_d = nc.dram_tensor("q_mask", [B, J], FP, kind="ExternalInput")
    w_d = nc.dram_tensor("w", [3 * D2], FP, kind="ExternalInput")
    b_d = nc.dram_tensor("b", [1, 1], FP, kind="ExternalInput")
    G_d = nc.dram_tensor("G", [B, T, DG], FP, kind="ExternalOutput")

    with tile.TileContext(nc) as tc:
        with (
            tc.tile_pool(name="const", bufs=1) as constp,
            tc.tile_pool(name="row", bufs=2) as rowp,
            tc.tile_pool(name="hrow", bufs=3) as hrowp,
            tc.tile_pool(name="chunk", bufs=8) as chp,
            tc.tile_pool(name="gbuf", bufs=3) as gp,
            tc.tile_pool(name="ps_tr", bufs=3, space="PSUM") as ps_trp,
            tc.tile_pool(name="ps_s", bufs=2, space="PSUM") as ps_sp,
            tc.tile_pool(name="ps_u", bufs=2, space="PSUM") as ps_up,
            tc.tile_pool(name="ps_sm", bufs=1, space="PSUM") as ps_smp,
        ):
            # ---- constants ----
            identb = constp.tile([128, 128], BF)
            make_identity(nc, identb)
            ident64 = constp.tile([64, 64], FP)
            make_identity(nc, ident64)
            ones_row = constp.tile([1, 128], BF)
            nc.vector.memset(ones_row, 1.0)
            ones_col = constp.tile([128, 1], BF)
            nc.vector.memset(ones_col, 1.0)
            ones2 = constp.tile([2, 128], BF)
            nc.vector.memset(ones2, 1.0)
            negc = constp.tile([128, 1], FP)
            nc.vector.memset(negc, -NEG_SOFT)
            b_sb = constp.tile([1, 1], FP)
            nc.gpsimd.dma_start(out=b_sb, in_=b_d[:, :])
            # w splits: wh/wu feed matmul rhs -> bf16 (cast on DMA);
            # whu feeds a DVE scalar operand -> fp32.
            wh1 = constp.tile([K1, 1], BF)
            wh2 = constp.tile([K2, 1], BF)
            wu1 = constp.tile([K1, 1], BF)
            wu2 = constp.tile([K2, 1], BF)
            whu1 = constp.tile([K1, 1], FP)
            whu2 = constp.tile([K2, 1], FP)
            for sb, lo in ((wh1, 0), (wh2, K1), (wu1, D2), (wu2, D2 + K1),
                           (whu1, 2 * D2), (whu2, 2 * D2 + K1)):
                n = sb.shape[0]
                nc.gpsimd.dma_start(out=sb, in_=w_d[lo:lo + n].unsqueeze(1))

            def row_setup(r):
                st = {"r": r}
                U_sb = rowp.tile([J, D2], FP, tag="usb")
                nc.sync.dma_start(out=U_sb, in_=U_d[r])
                qm_b = rowp.tile([128, J], FP, tag="qmb")
                nc.gpsimd.dma_start(out=qm_b, in_=qm_d[r].partition_broadcast(128))
                cm_t = rowp.tile([128, E], FP, tag="cmt")
                nc.gpsimd.dma_start(
                    out=cm_t, in_=cm_d[r].rearrange("(p e) -> p e", p=128)
                )
                Hb = hrowp.tile([128, E * D2], BF, tag="hb")
                Hv = H_d[r].rearrange("(p e) d -> p (e d)", p=128)
                nc.gpsimd.dma_start(out=Hb[:, 0:4 * D2], in_=Hv[:, 0:4 * D2])
                nc.gpsimd.dma_start(
                    out=Hb[:, 4 * D2:E * D2], in_=Hv[:, 4 * D2:E * D2]
                )
                # H part of G: cast-store straight from the bf16 H tile
                nc.gpsimd.dma_start(
                    out=G_d[r][:, 0:D2].rearrange("(p e) d -> p e d", p=128),
                    in_=Hb.rearrange("p (e d) -> p e d", d=D2),
                )

                # U in bf16 with a ones column (denominator trick)
                U_r = rowp.tile([J, D2 + 1], BF, tag="ur")
                nc.vector.tensor_copy(out=U_r[:, 0:D2], in_=U_sb)
                nc.gpsimd.memset(U_r[:, D2:D2 + 1], 1.0)

                # U^T (two d-blocks side by side) -> ut [128, 128]
                tru = ps_trp.tile([128, 256], BF, tag="tr", name="tru")
                nc.tensor.transpose(tru[:, 0:J], U_r[:, 0:K1], identb[0:J, 0:J])
                nc.tensor.transpose(
                    tru[0:K2, J:2 * J], U_r[:, K1:D2], identb[0:J, 0:J]
                )
                ut = rowp.tile([128, 2 * J], BF, tag="ut")
                nc.vector.tensor_copy(out=ut, in_=tru[:, 0:2 * J])

                # S-matmul rhs: [U^T*w_hu*qm | w_h] per d-block
                uwq1 = rowp.tile([K1, JS], BF, tag="uwq1")
                uwq2 = rowp.tile([K2, JS], BF, tag="uwq2")
                nc.vector.scalar_tensor_tensor(
                    out=uwq1[:, 0:J], in0=ut[:, 0:J], scalar=whu1[:, 0:1],
                    in1=qm_b[0:K1, :], op0=MULT, op1=MULT,
                )
                nc.vector.tensor_copy(out=uwq1[:, J:JS], in_=wh1)
                nc.vector.scalar_tensor_tensor(
                    out=uwq2[:, 0:J], in0=ut[0:K2, J:2 * J], scalar=whu2[:, 0:1],
                    in1=qm_b[0:K2, :], op0=MULT, op1=MULT,
                )
                nc.vector.tensor_copy(out=uwq2[:, J:JS], in_=wh2)

                # rank-1 rhs rows: [ (S2+b)*qm ; 100*qm ], S1 col = 0
                s2qm = rowp.tile([2, JS], BF, tag="s2qm")
                nc.vector.memset(s2qm, 0.0)
                ps2 = ps_smp.tile([J, 1], FP, tag="sm", name="ps2")
                nc.tensor.matmul(ps2, ut[:, 0:J], wu1, start=True, stop=False)
                nc.tensor.matmul(ps2, ut[0:K2, J:2 * J], wu2, start=False, stop=True)
                s2col = rowp.tile([J, 1], FP, tag="s2col")
                nc.vector.tensor_copy(out=s2col, in_=ps2)
                ps2r = ps_smp.tile([1, J], FP, tag="sm", name="ps2r")
                nc.tensor.transpose(ps2r, s2col, ident64)
                s2q = rowp.tile([1, J], BF, tag="s2q")
                nc.vector.scalar_tensor_tensor(
                    out=s2q, in0=ps2r, scalar=b_sb[:, 0:1],
                    in1=qm_b[0:1, :], op0=ADD, op1=MULT,
                )
                nc.sync.dma_start(out=s2qm[0:1, 0:J], in_=s2q)
                qm100 = rowp.tile([1, J], BF, tag="qm100")
                nc.vector.tensor_scalar_mul(
                    out=qm100, in0=qm_b[0:1, :], scalar1=NEG_SOFT
                )
                nc.sync.dma_start(out=s2qm[1:2, 0:J], in_=qm100)

                st["Hb"], st["U_r"], st["qm_b"], st["cm_t"] = Hb, U_r, qm_b, cm_t
                st["uwq1"], st["uwq2"], st["s2qm"] = uwq1, uwq2, s2qm
                st["g"] = gp.tile([128, E * DGR], FP, tag="g", name="g")
                st["ps_half"] = [None, None]
                st["e_pair"] = [None] * (E // 2)
                st["maxes"] = rowp.tile([128, E], FP, tag="maxes", name="maxes")
                st["expS1"] = rowp.tile([128, E], FP, tag="es1", name="expS1")
                st["ps_up"] = [None] * (E // 2)
                st["rp"] = [None] * (E // 2)
                return st

            def head(st, e):
                # logits chunk: St*qm in PSUM cols 0:64, S1 in col 64
                h2, idx = e // 4, e % 4
                if idx == 0:
                    st["ps_half"][h2] = ps_sp.tile(
                        [128, 4 * JS], FP, tag="srow", name="ps_half"
                    )
                ps = st["ps_half"][h2]
                Hb = st["Hb"]
                trc = ps_trp.tile([128, 256], BF, tag="tr", name="trc")
                nc.tensor.transpose(trc[:, 0:128], Hb[:, e * D2:e * D2 + K1], identb)
                nc.tensor.transpose(
                    trc[0:K2, 128:256], Hb[:, e * D2 + K1:(e + 1) * D2], identb
                )
                ht = chp.tile([128, 256], BF, tag="ht")
                nc.scalar.copy(out=ht, in_=trc)
                cols = slice(idx * JS, (idx + 1) * JS)
                nc.tensor.matmul(
                    ps[:, cols], ht[:, 0:128], st["uwq1"], start=True, stop=False
                )
                nc.tensor.matmul(
                    ps[:, cols], ht[0:K2, 128:256], st["uwq2"],
                    start=False, stop=False,
                )
                nc.tensor.matmul(
                    ps[:, cols], ones2, st["s2qm"], start=False, stop=True
                )

            def exppair(st, h):
                # exp for one chunk-pair; S1 exp once per half (after pair 1/3)
                h2 = h // 2
                ps = st["ps_half"][h2]
                lo = (2 * h % 4) * JS
                e_pair = chp.tile([128, 2 * JS], BF, tag="eh")
                st["e_pair"][h] = e_pair
                nc.scalar.activation(
                    out=e_pair, in_=ps[:, lo:lo + 2 * JS], func=EXP,
                    bias=negc[:, 0:1], scale=1.0,
                )
                nc.vector.reduce_max(
                    st["maxes"][:, 2 * h:2 * h + 2],
                    e_pair.rearrange("p (c x) -> p c x", x=JS),
                    axis=AXX,
                )
                if h % 2 == 1:
                    ps3 = ps.rearrange("p (c x) -> p c x", x=JS)
                    nc.scalar.activation(
                        out=st["expS1"][:, h2 * 4:(h2 + 1) * 4],
                        in_=ps3[:, :, J:JS], func=EXP,
                    )

            def pair(st, h):
                # chunks (2h, 2h+1): transpose e' pair, U_ matmuls + denom
                e_pair = st["e_pair"][h]
                eTps = ps_trp.tile([J, 256], BF, tag="tr", name="eTps")
                for k in range(2):
                    nc.tensor.transpose(
                        eTps[:, k * 128:(k + 1) * 128],
                        e_pair[:, k * JS:k * JS + J], identb,
                    )
                eT = chp.tile([J, 256], BF, tag="eT")
                nc.vector.tensor_copy(out=eT, in_=eTps)
                ps_u = ps_up.tile([128, 2 * (D2 + 1)], FP, tag="up")
                st["ps_up"][h] = ps_u
                nc.tensor.matmul(
                    ps_u[:, 0:D2 + 1], eT[:, 0:128], st["U_r"],
                    start=True, stop=True,
                )
                nc.tensor.matmul(
                    ps_u[:, D2 + 1:2 * (D2 + 1)], eT[:, 128:256], st["U_r"],
                    start=True, stop=True,
                )
                rp = chp.tile([128, 2], FP, tag="rp")
                st["rp"][h] = rp
                nc.vector.reciprocal(
                    out=rp,
                    in_=ps_u.rearrange("p (c x) -> p c x", x=D2 + 1)[:, :, D2:D2 + 1],
                )

            def tail(st, c):
                h, half = c // 2, c % 2
                ps_u, rp, g = st["ps_up"][h], st["rp"][h], st["g"]
                lo = half * (D2 + 1)
                # U_ = (e'@U)/denom, normalization folded into the ACT copy
                nc.scalar.activation(
                    out=g[:, c * DGR:c * DGR + D2],
                    in_=ps_u[:, lo:lo + D2], func=COPYF,
                    scale=rp[:, half:half + 1],
                )
                nc.vector.tensor_tensor(
                    out=g[:, c * DGR + D2:c * DGR + 2 * D2],
                    in0=st["Hb"][:, c * D2:(c + 1) * D2],
                    in1=g[:, c * DGR:c * DGR + D2], op=MULT,
                )

            def rowend(st):
                # rt = exp(S1) * max_j(e') * cm ; H_ = (rt @ H) / sum(rt)
                rt = rowp.tile([128, E], FP, tag="rt")
                nc.vector.tensor_tensor(
                    out=rt, in0=st["maxes"], in1=st["expS1"], op=MULT
                )
                rtb = rowp.tile([128, E], BF, tag="rtb")
                nc.vector.tensor_tensor(
                    out=rtb, in0=rt, in1=st["cm_t"], op=MULT
                )
                Hb = st["Hb"]
                smt = ps_smp.tile([128, D2 + E], FP, tag="sm", name="smt")
                hbar = smt[0:1, 0:D2]
                for e in range(E):
                    nc.tensor.matmul(
                        hbar, rtb[:, e:e + 1], Hb[:, e * D2:(e + 1) * D2],
                        start=(e == 0), stop=(e == E - 1),
                    )
                rsum = smt[0:1, D2:D2 + E]
                nc.tensor.matmul(rsum, ones_col, rtb, start=True, stop=True)
                rsum_sb = rowp.tile([1, E], FP, tag="rsumsb")
                nc.vector.tensor_copy(out=rsum_sb, in_=rsum)
                rs = rowp.tile([1, 1], FP, tag="rs")
                nc.vector.reduce_sum(rs, rsum_sb, axis=AXX)
                nc.vector.reciprocal(out=rs, in_=rs)
                hbar_sb = rowp.tile([1, D2], BF, tag="hbarsb")
                nc.vector.tensor_scalar_mul(
                    out=hbar_sb, in0=hbar, scalar1=rs[:, 0:1]
                )
                ps_hb = smt[:, 0:D2]
                nc.tensor.matmul(ps_hb, ones_row, hbar_sb, start=True, stop=True)
                hb_sb = rowp.tile([128, D2], BF, tag="hbsb")
                nc.vector.tensor_copy(out=hb_sb, in_=ps_hb)
                st["hb_sb"] = hb_sb

            def rowfin(st, e):
                # H*H_ for one chunk (Pool keeps DVE free)
                g, Hb = st["g"], st["Hb"]
                nc.gpsimd.tensor_mul(
                    g[:, e * DGR + 2 * D2:e * DGR + 3 * D2],
                    Hb[:, e * D2:(e + 1) * D2], st["hb_sb"]
                )

            def rowstore(st, h2):
                # U_|HU|HH part of G, plain fp32 on the HWDGE sync queue;
                # stored in half-rows so the tail drains sooner
                es = slice(h2 * 4, (h2 + 1) * 4)
                nc.sync.dma_start(
                    out=G_d[st["r"]].rearrange(
                        "(p e) g -> p e g", p=128
                    )[:, es, D2:DG],
                    in_=st["g"].rearrange("p (e g) -> p e g", g=DGR)[:, es, :],
                )

            # ---- cross-row pipelined schedule ----
            states = [None] * B
            states[0] = row_setup(0)
            for e in range(E):
                head(states[0], e)
            for r in range(B):
                st = states[r]
                prev = states[r - 1] if r > 0 else None
                if r + 1 < B:
                    states[r + 1] = row_setup(r + 1)
                for h in range(E // 2):
                    exppair(st, h)
                    pair(st, h)
                    tail(st, 2 * h)
                    tail(st, 2 * h + 1)
                    if prev is not None:
                        rowfin(prev, 2 * h)
                        rowfin(prev, 2 * h + 1)
                    if r + 1 < B:
                        head(states[r + 1], 2 * h)
                        head(states[r + 1], 2 * h + 1)
                    if prev is not None and h % 2 == 1:
                        rowstore(prev, h // 2)
                rowend(st)
            for h2 in range(2):
                for e in range(4 * h2, 4 * h2 + 4):
                    rowfin(states[B - 1], e)
                rowstore(states[B - 1], h2)

    if split_waits:
        _split_overwide_waits(nc)
    return nc


_NC_CACHE = None


def _get_nc():
    global _NC_CACHE
    if _NC_CACHE is None:
        _NC_CACHE = build_program()
    return _NC_CACHE


def run_sharded(inputs, trace=False):
    from concourse.bass_utils import run_bass_kernel_spmd

    H = np.ascontiguousarray(np.asarray(inputs["H"], dtype=np.float32))
    U = np.ascontiguousarray(np.asarray(inputs["U"], dtype=np.float32))
    cm = np.ascontiguousarray(np.asarray(inputs["c_mask"], dtype=np.float32))
    qm = np.ascontiguousarray(np.asarray(inputs["q_mask"], dtype=np.float32))
    w = np.ascontiguousarray(np.asarray(inputs["w"], dtype=np.float32))
    b = np.asarray(inputs["b"], dtype=np.float32).reshape(1, 1)

    nc = _get_nc()
    in_maps = []
    for c in range(N_CORES):
        s = slice(c * B, (c + 1) * B)
        in_maps.append(
            {"H": H[s], "U": U[s], "c_mask": cm[s], "q_mask": qm[s], "w": w, "b": b}
        )
    res = run_bass_kernel_spmd(
        nc, in_maps, core_ids=list(range(N_CORES)), trace=trace
    )
    G = np.concatenate([res.results[c]["G"] for c in range(N_CORES)], axis=0)
    return G, res


def kernel(H, U, c_mask, q_mask, w, b):
    G, _ = run_sharded(
        {"H": H, "U": U, "c_mask": c_mask, "q_mask": q_mask, "w": w, "b": b}
    )
    return G
